# revision 42
# baseline (speedup 1.0000x reference)
"""Trainium2 Bass kernel for CBSA (cross-block self-attention) module — v3.

Shapes (hardcoded from the problem spec):
  x: [8, 4096, 512], proj_w/to_out_w: [512, 512], step_rep/step_x: [8,1,1],
  to_out_b: [512].  Output: [8, 4096, 512].

Sharding: data-parallel over batch, 1 batch per NeuronCore (8 cores).

v3 design notes (driven by the baseline perfetto trace):
 - Input streamed in 512-col slices so the first matmul starts ~4us earlier.
 - Pooling matmuls folded into the phase-1 GEMM stream (no serial pool step).
 - wT stored NATURAL ([128, 4, N], d = 128*di + k); dots run non-DR fp8 with
   a dense block-diag lhsT (FWL weight loads), writing q-natural PSUM rows
   that cast straight into the DR-interleaved edf8 layout phase 6 needs.
 - exp() in 1024-col chunks (2 PSUM banks per activation), no accum_out:
   softmax row sums come from GPSIMD tensor_reduce over the fp8 ed rows
   (Pool engine is otherwise idle; ACT was the phase-3 bottleneck).
 - Per-pair chains issued STAGE-major (all rd, then all rp, ...) so the
   cross-engine latency pipelines across pairs instead of serializing.
 - Phase-6 PSUM pool widened; output DMAs alternate sync/gpsimd queues.
"""

import numpy as np
import ml_dtypes

import concourse.bass as bass
import concourse.tile as tile
from concourse import bacc, mybir
from concourse import bass_utils

F32 = mybir.dt.float32
BF16 = mybir.dt.bfloat16
FP8 = mybir.dt.float8e4

B = 8
N = 4096
C = 512
HEADS = 8
DH = 64
Q = 64            # pooled tokens
SCALE = DH ** -0.5
NT = N // 128     # 32 token tiles
NU = NT // 2      # 16 token tile-pairs (DoubleRow k-tiles)
PAIRS = HEADS // 2  # 4 head pairs
NS = N // 512     # 8 free-dim slices of 512
MSC = 2.0 ** 14   # fp8 scaling for M (xds carries 1/(s1*s2) which is ~1e-4)

_CACHE = {}


def _build():
    nc = bacc.Bacc("TRN2", target_bir_lowering=False, debug=False, num_devices=B)

    xT_d = nc.dram_tensor("xT", [128, 2, 2, N], FP8, kind="ExternalInput").ap()
    pwT_d = nc.dram_tensor("pwT", [128, 2, 2, C], FP8, kind="ExternalInput").ap()
    poolT_d = nc.dram_tensor("poolT", [128, NU, 2, Q], FP8,
                             kind="ExternalInput").ap()
    twp_d = nc.dram_tensor("twp", [128, PAIRS, C], BF16,
                           kind="ExternalInput").ap()
    srep_d = nc.dram_tensor("srep", [128, PAIRS], F32, kind="ExternalInput").ap()
    bcc_d = nc.dram_tensor("biascc", [128, 4], F32, kind="ExternalInput").ap()
    idb_d = nc.dram_tensor("identb", [128, 128], BF16, kind="ExternalInput").ap()
    idf_d = nc.dram_tensor("identf", [128, 128], F32, kind="ExternalInput").ap()
    outT_d = nc.dram_tensor("outT", [C, N], BF16, kind="ExternalOutput").ap()

    from contextlib import ExitStack
    with tile.TileContext(nc) as tc:
        with ExitStack() as ctx:
            _body.ctx = ctx
            _body(tc, nc, xT_d, pwT_d, poolT_d, twp_d, srep_d, bcc_d,
                  idb_d, idf_d, outT_d)
    nc.compile()
    return nc


def _body(tc, nc, xT_d, pwT_d, poolT_d, twp_d, srep_d, bcc_d,
          idb_d, idf_d, outT_d):
    Exp = mybir.ActivationFunctionType.Exp
    Copy = mybir.ActivationFunctionType.Copy
    X = mybir.AxisListType.X
    ADD = mybir.AluOpType.add
    MULT = mybir.AluOpType.mult
    DR = mybir.MatmulPerfMode.DoubleRow

    ctx = _body.ctx
    const = ctx.enter_context(tc.tile_pool(name="const", bufs=1))
    persist = ctx.enter_context(tc.tile_pool(name="persist", bufs=1))
    xs_pool = ctx.enter_context(tc.tile_pool(name="xstream", bufs=3))
    sm = ctx.enter_context(tc.tile_pool(name="small", bufs=2))
    ost = ctx.enter_context(tc.tile_pool(name="ostage", bufs=6))
    from contextlib import ExitStack as _ES

    # ---- constants (pwT first: phase-1 lhsT/rhs; poolT on gpsimd queue;
    # everything else is queued on sync AFTER the x stream) ----
    pwT = const.tile([128, 2, 2, C], FP8, tag="pwT")
    nc.scalar.dma_start(pwT[:], pwT_d[:])
    poolT = const.tile([128, NU, 2, Q], FP8, tag="poolT")
    nc.gpsimd.dma_start(poolT[:], poolT_d[:])
    identb = const.tile([128, 128], BF16, tag="identb")
    identf = const.tile([128, 128], F32, tag="identf")
    srep = const.tile([128, PAIRS], F32, tag="srep")
    twp = const.tile([128, PAIRS, C], BF16, tag="twp")
    biascc = const.tile([128, 4], F32, tag="biascc")

    # ---- persistent intermediates ----
    # wT natural fp8: wTn[k, di, n] holds d = 128*di + k (pair p == di)
    wTn = persist.tile([128, 4, N], FP8, tag="wTn", name="wTn")
    # w natural fp8, n-interleaved: wN[k, u, j, d] holds n = 256u + 2k + j
    # (matches the byte-pair layout the DMA-transposed attn rows come in)
    wN = persist.tile([128, NU, 2, C], FP8, tag="wN", name="wN")
    # exp(dots): edf8[g][k_q, j_p, n]  (pair p = 2g + j_p; k_q = 64*h_loc + q)
    edf8 = [persist.tile([128, 2, N], FP8, tag=f"ed{g}", name=f"ed{g}")
            for g in range(2)]
    # exp(dots)^T via one DMA xbar transpose per pair, on the bf16 view:
    # at2b[p][k, u, q] (bf16) <=> fp8 [k, u, q, b] with n = 256u + 2k + b
    at2b = [persist.tile([128, NU, 128], BF16, tag=f"at2{p}", name=f"at2{p}")
            for p in range(PAIRS)]
    at2v = [t[:].bitcast(FP8).rearrange("p u (q b) -> p u b q", b=2)
            for t in at2b]
    # M (to_out-folded pooled outputs) fp8, scaled by MSC: Mf8[g][k_q, j_p, c]
    Mf8 = [persist.tile([128, 2, C], FP8, tag=f"Mf8{g}", name=f"Mf8{g}")
           for g in range(2)]

    # block-diag dots lhsT / chain tiles, memset early while engines are idle
    dblk, rnat, ed2 = [], [], []
    for p in range(PAIRS):
        bk = sm.tile([128, 128], FP8, tag=f"dblk{p}")
        nc.gpsimd.memset(bk[:], 0.0)
        dblk.append(bk)
        rn = sm.tile([128, 128], BF16, tag=f"rnat{p}")
        nc.gpsimd.memset(rn[:], 0.0)
        rnat.append(rn)
        e2 = sm.tile([128, 128], BF16, tag=f"ed2_{p}")
        nc.gpsimd.memset(e2[:], 0.0)
        ed2.append(e2)

    # ============ Phase A: wT / wN GEMMs + pooling, streamed per 512 cols ==
    pp_ps = _ES()
    ppool = pp_ps.enter_context(
        tc.tile_pool(name="pp", bufs=1, space="PSUM"))
    phA_ps = _ES()
    ps512 = phA_ps.enter_context(
        tc.tile_pool(name="ps1", bufs=6, space="PSUM"))
    # pooled rep accumulator [64 q, 512 d-natural]
    psp = ppool.tile([64, C], F32, tag="psp", name="psp")
    prp_t = ppool.tile([128, 4, Q], F32, tag="prp", name="prp")

    cast_ctr = [0]

    def cast(dst, src, scale, eng=None):
        # alternate PSUM->SBUF cast streams between ACT and DVE
        if eng is None:
            eng = "a" if cast_ctr[0] % 2 == 0 else "v"
            cast_ctr[0] += 1
        if eng == "a":
            nc.scalar.activation(dst, src, Copy, scale=scale)
        else:
            nc.vector.tensor_scalar_mul(dst, src, scale)

    def pool_mms(u):
        nc.tensor.matmul(psp[:], poolT[:, u, :, :], wN[:, u, :, :],
                         start=(u == 0), stop=(u == NU - 1), perf_mode=DR)

    def wT_mms(sl, eng=None):
        for di in range(4):
            pst = ps512.tile([128, 512], F32, tag="ps512", name="pst")
            for g in range(2):
                nc.tensor.matmul(
                    pst[:],
                    pwT[:, g, :, di * 128:(di + 1) * 128],
                    xts[sl][:, g, :, :],
                    start=(g == 0), stop=(g == 1),
                    perf_mode=DR)
            cast(wTn[:, di, sl * 512:(sl + 1) * 512], pst[:], 1.0 / 16.0,
                 eng=eng)

    def wN_mms(sl):
        # psum rows = the n-interleaved set {256u + 2k + j}: lhsT columns are
        # a stride-2 selection of the slice's token columns
        for u2l in range(2):
            u = sl * 2 + u2l
            xv = xts[sl][:, :, :, u2l * 256:(u2l + 1) * 256].rearrange(
                "p g j (n b) -> p g j b n", b=2)
            for j in range(2):
                wps = ps512.tile([128, 512], F32, tag="ps512", name="wps")
                for g in range(2):
                    nc.tensor.matmul(
                        wps[:],
                        xv[:, g, :, j, :],
                        pwT[:, g, :, :],
                        start=(g == 0), stop=(g == 1),
                        perf_mode=DR)
                cast(wN[:, u, j, :], wps[:], 1.0 / 16.0)

    xts = {}
    for sl in range(NS):
        xts[sl] = xs_pool.tile([128, 2, 2, 512], FP8, tag="xs", name="xts")
        nc.sync.dma_start(xts[sl][:], xT_d[:, :, :, sl * 512:(sl + 1) * 512])
        # pool matmuls for the previous slice's token tiles (casts done)
        if 1 <= sl <= NS - 2:
            pool_mms(2 * (sl - 1))
            pool_mms(2 * (sl - 1) + 1)
        if sl < NS - 2:
            wT_mms(sl)
            wN_mms(sl)
    # tail: finish all wN + pooling first; the two remaining wT slices are
    # issued after the rep chain below so they cover its serial latency
    wN_mms(NS - 2)
    wN_mms(NS - 1)
    pool_mms(NU - 4)
    pool_mms(NU - 3)
    pool_mms(NU - 2)
    pool_mms(NU - 1)

    # deferred constants, queued on sync behind the x stream
    nc.sync.dma_start(identf[:], idf_d[:])
    nc.sync.dma_start(identb[:], idb_d[:])
    nc.sync.dma_start(srep[:], srep_d[:])
    nc.sync.dma_start(twp[:], twp_d[:])
    nc.sync.dma_start(biascc[:], bcc_d[:])

    # ============ Phase B: repT + block-diag dots lhsT ====================
    rep_sb = sm.tile([64, C], F32, tag="rep_sb")
    nc.vector.tensor_copy(rep_sb[:], psp[:])
    # rep rows for odd heads, shifted to partitions 64:128 via SBUF DMA
    # (engines cannot move data across partitions; DMA can, and it's idle)
    rep_hi = sm.tile([128, 4, Q], F32, tag="rep_hi")
    for p in range(PAIRS):
        nc.sync.dma_start(rep_hi[64:128, p, :],
                          rep_sb[0:64, p * 128 + 64:p * 128 + 128])

    # last two wT slices run while the rep chain drains; their casts go to
    # DVE only so the ACT queue is clear for the dblk copies gating dots
    wT_mms(NS - 2, eng="v")
    wT_mms(NS - 1, eng="v")

    # repT[k, di, q] = rep[q, 128*di + k] via f32 PE transposes
    for di in range(4):
        nc.tensor.transpose(prp_t[:, di, :],
                            rep_sb[:, di * 128:(di + 1) * 128],
                            identf[0:64, 0:64])

    # block-diag lhsT for dots, fp8 natural, cast straight from PSUM on ACT
    # (whose queue is idle here; DVE carries the deferred wT casts)
    for p in range(PAIRS):
        bk = dblk[p]
        nc.scalar.copy(bk[0:64, 0:64], prp_t[0:64, p, :])
        nc.scalar.copy(bk[64:128, 64:128], prp_t[64:128, p, :])

    phA_ps.close()
    pp_ps.close()

    # ============ Phase C: dots + exp + DMA transposes + rep_delta ========
    phD1_ps = _ES()
    rd_all = phD1_ps.enter_context(
        tc.tile_pool(name="rdp", bufs=1, space="PSUM")).tile(
        [128, 4, 128], F32, tag="rd_all", name="rd_all")
    phC_ps = _ES()
    psd = phC_ps.enter_context(
        tc.tile_pool(name="psd", bufs=3, space="PSUM"))

    # dots for all pairs first (PE stream leads ACT's exp stream); the
    # attn^T transposes fire as DMA xbar ops on bf16 views, per 2048-col
    # half so each becomes available right after its exp chunks, all on
    # the otherwise-idle sync queue (scalar queue would stall exp)
    s1parts = []
    for p in range(PAIRS):
        g, pl = p // 2, p % 2
        s1p = sm.tile([128, 4], F32, tag=f"s1parts{p}")
        for sl2 in range(4):
            dps = psd.tile([128, 1024], F32, tag="ps1024", name="dps")
            for h in range(2):
                c0 = sl2 * 1024 + h * 512
                nc.tensor.matmul(dps[:, h * 512:(h + 1) * 512],
                                 dblk[p][:],
                                 wTn[:, p, c0:c0 + 512],
                                 start=True, stop=True)
            nc.scalar.activation(edf8[g][:, pl, sl2 * 1024:(sl2 + 1) * 1024],
                                 dps[:], Exp, scale=SCALE,
                                 accum_out=s1p[:, sl2:sl2 + 1])
            if sl2 % 2 == 1:
                hh = sl2 // 2
                nc.sync.dma_start_transpose(
                    at2b[p][:, hh * 8:(hh + 1) * 8, :],
                    edf8[g][:, pl, hh * 2048:(hh + 1) * 2048].bitcast(BF16))
        s1parts.append(s1p)

    # softmax row sums: reduce the per-chunk accumulators (DVE, tiny)
    rc1, ssc = [], []
    for p in range(PAIRS):
        s1p = sm.tile([128, 1], F32, tag=f"s1_{p}")
        nc.vector.tensor_reduce(s1p[:], s1parts[p][:], X, ADD)
        rc = sm.tile([128, 1], F32, tag=f"rc1_{p}")
        nc.vector.reciprocal(rc[:], s1p[:])
        sscp = sm.tile([128, 1], F32, tag=f"ssc_{p}")
        nc.vector.tensor_mul(sscp[:], rc[:], srep[:, p:p + 1])
        rc1.append(rc)
        ssc.append(sscp)

    # rep_delta accumulation off the DMA-transposed attn rows. Per byte
    # lane b, lhsT partition k holds n = 256u + 2k + b — exactly wN's
    # j = b lane, so plain (non-DR) matmuls line up with no repacking.
    for p in range(PAIRS):
        for u in range(NU):
            for b in range(2):
                nc.tensor.matmul(rd_all[:, p, :], at2v[p][:, u, b, :],
                                 wN[:, u, b, p * 128:(p + 1) * 128],
                                 start=(u == 0 and b == 0),
                                 stop=(u == NU - 1 and b == 1))

    phC_ps.close()

    # ============ Phase D: per-pair chains, STAGE-major ===================
    phD_ps = _ES()
    rtp_all = phD_ps.enter_context(
        tc.tile_pool(name="rtpp", bufs=1, space="PSUM")).tile(
        [128, 4, 128], BF16, tag="rtp_all", name="rtp_all")
    d2_all = phD_ps.enter_context(
        tc.tile_pool(name="d2p", bufs=1, space="PSUM")).tile(
        [128, 4, 128], F32, tag="d2_all", name="d2_all")
    xdt_all = phD_ps.enter_context(
        tc.tile_pool(name="xdtp", bufs=1, space="PSUM")).tile(
        [128, 4, 128], F32, tag="xdt_all", name="xdt_all")
    mpool = phD_ps.enter_context(
        tc.tile_pool(name="mps", bufs=2, space="PSUM"))

    # stage 2: reph_new (natural, block-diag) bf16; rep comes from
    # rep_sb (even heads, partitions 0:64) / rep_hi (odd heads, 64:128)
    for p in range(PAIRS):
        nc.vector.scalar_tensor_tensor(rnat[p][0:64, 0:64],
                                       rd_all[0:64, p, 0:64],
                                       ssc[p][0:64, 0:1],
                                       rep_sb[0:64, p * 128:p * 128 + 64],
                                       MULT, ADD)
        nc.vector.scalar_tensor_tensor(rnat[p][64:128, 64:128],
                                       rd_all[64:128, p, 64:128],
                                       ssc[p][64:128, 0:1],
                                       rep_hi[64:128, p, :],
                                       MULT, ADD)
    # stage 4: reph_new^T
    rnT = []
    for p in range(PAIRS):
        nc.tensor.transpose(rtp_all[:, p, :], rnat[p][:], identb[:])
    for p in range(PAIRS):
        rT = sm.tile([128, 128], BF16, tag=f"rnT{p}")
        nc.scalar.copy(rT[:], rtp_all[:, p, :])
        rnT.append(rT)
    # stage 5: dots2 (block-diag) + exp; row sums on DVE off the bf16 tile
    for p in range(PAIRS):
        nc.tensor.matmul(d2_all[:, p, :], rnT[p][:], rnT[p][:],
                         start=True, stop=True)
    s2 = []
    for p in range(PAIRS):
        e2 = ed2[p]
        s2p = sm.tile([128, 1], F32, tag=f"s2_{p}")
        for h in range(2):
            r0, r1 = 64 * h, 64 * (h + 1)
            nc.scalar.activation(e2[r0:r1, r0:r1], d2_all[r0:r1, p, r0:r1],
                                 Exp, scale=SCALE)
        for h in range(2):
            r0, r1 = 64 * h, 64 * (h + 1)
            nc.vector.tensor_reduce(s2p[r0:r1, 0:1], e2[r0:r1, r0:r1],
                                    X, ADD)
        s2.append(s2p)
    # stage 6: xds^T directly — ed2 is SYMMETRIC (d2 = rnT.T@rnT is), so
    # xds^T = rnat^T @ ed2 needs no transpose; softmax normalization moves
    # into the M cast as a per-partition (pooled-q) scale.
    for p in range(PAIRS):
        nc.tensor.matmul(xdt_all[:, p, :], rnat[p][:], ed2[p][:],
                         start=True, stop=True)
    xdsT, fS = [], []
    for p in range(PAIRS):
        rc2 = sm.tile([128, 1], F32, tag=f"rc2_{p}")
        nc.vector.reciprocal(rc2[:], s2[p][:])
        fs = sm.tile([128, 1], F32, tag=f"fS_{p}")
        nc.vector.tensor_scalar(fs[:], rc2[:], rc1[p][:], MSC, MULT, MULT)
        fS.append(fs)
        xT_sb = sm.tile([128, 128], BF16, tag=f"xdsT{p}")
        if p % 2 == 0:
            nc.scalar.copy(xT_sb[:], xdt_all[:, p, :])
        else:
            nc.vector.tensor_copy(xT_sb[:], xdt_all[:, p, :])
        xdsT.append(xT_sb)
    # stage 7: M_pair = xds^T.T @ twp_pair, scaled by MSC/(s1*s2) per q-row
    for p in range(PAIRS):
        g, pl = p // 2, p % 2
        mps = mpool.tile([128, 512], F32, tag="mps", name="mps")
        nc.tensor.matmul(mps[:], xdsT[p][:], twp[:, p, :],
                         start=True, stop=True)
        if p % 2 == 0:
            nc.scalar.activation(Mf8[g][:, pl, :], mps[:], Copy,
                                 scale=fS[p][:, 0:1])
        else:
            nc.vector.tensor_scalar_mul(Mf8[g][:, pl, :], mps[:], fS[p][:, 0:1])

    phD_ps.close()
    phD1_ps.close()
    phE_ps = _ES()
    pso = phE_ps.enter_context(
        tc.tile_pool(name="pso", bufs=8, space="PSUM"))

    # ============ Phase E: out^T = sum_g M_g^T @ ed_g + bias ==============
    outv = outT_d.rearrange("(a p) n -> p a n", p=128)
    dma_engs = [nc.gpsimd, nc.sync, nc.scalar]
    dctr = 0
    for ci in range(4):
        for s2 in range(NS // 2):
            ot = ost.tile([128, 2, 512], BF16, tag="ostage", name="ot")
            for k in range(2):
                s = s2 * 2 + k
                ops = pso.tile([128, 512], F32, tag="ps512o", name="ops")
                for g in range(2):
                    nc.tensor.matmul(ops[:],
                                     Mf8[g][:, :, ci * 128:(ci + 1) * 128],
                                     edf8[g][:, :, s * 512:(s + 1) * 512],
                                     start=(g == 0), stop=(g == 1),
                                     perf_mode=DR)
                if (ci * 2 + k) % 2 == 0:
                    nc.scalar.activation(ot[:, k, :], ops[:],
                                         mybir.ActivationFunctionType.Identity,
                                         scale=1.0 / MSC,
                                         bias=biascc[:, ci:ci + 1])
                else:
                    nc.vector.tensor_scalar(ot[:, k, :], ops[:], 1.0 / MSC,
                                            biascc[:, ci:ci + 1], MULT, ADD)
                dma_engs[dctr % 3].dma_start(
                    outv[:, ci, s * 512:(s + 1) * 512], ot[:, k, :])
                dctr += 1


def _prep_inputs(x, proj_w, step_rep, step_x, to_out_w, to_out_b):
    x = np.asarray(x, dtype=np.float32)
    proj_w = np.asarray(proj_w, dtype=np.float32)
    step_rep = np.asarray(step_rep, dtype=np.float32).reshape(HEADS)
    step_x = np.asarray(step_x, dtype=np.float32).reshape(HEADS)
    to_out_w = np.asarray(to_out_w, dtype=np.float32)
    to_out_b = np.asarray(to_out_b, dtype=np.float32)

    # pwT: [k, g, j, d-col] fp8, c = 256g + 2k + j, free cols = natural d
    pw16 = (proj_w.T * 16.0).reshape(2, 128, 2, C)
    pwT = np.ascontiguousarray(pw16.transpose(1, 0, 2, 3)).astype(
        ml_dtypes.float8_e4m3)

    # poolT: [k, u, j, q], 1/64 where token n = 256u + 2k + j is in cell q
    n_idx = (256 * np.arange(NU)[None, :, None]
             + 2 * np.arange(128)[:, None, None]
             + np.arange(2)[None, None, :])              # [128, NU, 2]
    q_idx = (n_idx // 512) * 8 + (n_idx % 64) // 8       # cell index
    poolT = np.zeros((128, NU, 2, Q), dtype=np.float32)
    np.put_along_axis(poolT, q_idx[..., None], 1.0 / 64.0, axis=3)
    poolT = poolT.astype(ml_dtypes.float8_e4m3)

    # twp: [r, p, c] with pair p's block = natural d rows p*128 .. p*128+128
    twTs = to_out_w.T * np.repeat(step_x, DH)[:, None]   # [d_global, c_out]
    twp = np.ascontiguousarray(
        twTs.reshape(PAIRS, 128, C).transpose(1, 0, 2)).astype(
        ml_dtypes.bfloat16)

    biascc = np.ascontiguousarray(
        to_out_b.reshape(4, 128).T.astype(np.float32))
    srep = np.empty((128, PAIRS), dtype=np.float32)
    for p in range(PAIRS):
        srep[0:64, p] = step_rep[2 * p]
        srep[64:128, p] = step_rep[2 * p + 1]

    identb = np.eye(128, dtype=ml_dtypes.bfloat16)
    identf = np.eye(128, dtype=np.float32)

    shared = {
        "pwT": pwT, "poolT": poolT, "twp": twp, "srep": srep,
        "biascc": biascc, "identb": identb, "identf": identf,
    }
    in_maps = []
    for b in range(B):
        xT = np.ascontiguousarray(
            x[b].T.reshape(2, 128, 2, N).transpose(1, 0, 2, 3)).astype(
            ml_dtypes.float8_e4m3)
        in_maps.append({"xT": xT, **shared})
    return in_maps


def kernel(x, proj_w, step_rep, step_x, to_out_w, to_out_b):
    if "nc" not in _CACHE:
        _CACHE["nc"] = _build()
    nc = _CACHE["nc"]
    in_maps = _prep_inputs(x, proj_w, step_rep, step_x, to_out_w, to_out_b)
    res = bass_utils.run_bass_kernel_spmd(nc, in_maps, core_ids=list(range(B)))
    return np.stack(
        [np.asarray(res.results[b]["outT"]).astype(np.float32).T
         for b in range(B)], axis=0)
